# revision 1
# baseline (speedup 1.0000x reference)
"""Causal self-attention (RoPE on k/v) TRN2 Bass kernel.

Sharding: core i handles batch b = i//2 and 8 heads hs = 8*(i%2).
Each core computes qkv projection for its (batch, head-group), RoPE on
k and v, causal attention, and a partial output projection y^T with its
W_proj row-block.  Host sums the two partials per batch and adds b_proj.

Layouts on device (per core):
  xT   [C, T]     x[b]^T (host-transposed)
  qkT  [2048, T]  rows 0-1023 = q^T (head-major, perm'd d), 1024-2047 = rope(k)^T
  vr   [T, 1024]  rope(v), natural layout
  oT   [1024, T]  attention out^T
  yT   [C, T]     partial out-proj (output)

Head-dim permutation (even dims first) turns RoPE's interleaved
even/odd pairs into contiguous 64-row/col halves; W_attn columns and
W_proj rows are permuted correspondingly on host, which leaves the
attention math invariant.

All matmul operands are float32r (fp32 bits, 1 cycle/row on PE at
N>=256 vs 4 for plain fp32; measured relerr 1.5e-4 vs fp64, identical
to the fp32 matmul path on this HW).
"""
import sys

sys.path.insert(0, "/opt/trn_rl_repo")

import numpy as np

import concourse.bass as bass  # noqa: F401
import concourse.mybir as mybir
import concourse.tile as tile
from concourse import bacc
from concourse.bass_utils import run_bass_kernel_spmd

B, T, C, H = 4, 2048, 2048, 16
HD = 128
HC = 8  # heads per core
NCORES = 8
F32 = mybir.dt.float32
F32R = mybir.dt.float32r
SCALE = float(1.0 / np.sqrt(HD))

_CACHE = {}


def _build_nc():
    nc = bacc.Bacc(num_devices=NCORES)

    xT = nc.dram_tensor("xT", [C, T], F32R, kind="ExternalInput")
    wqk = nc.dram_tensor("wqk", [C, 2048], F32R, kind="ExternalInput")
    bqk = nc.dram_tensor("bqk", [128, 16], F32, kind="ExternalInput")
    wv = nc.dram_tensor("wv", [C, 1024], F32R, kind="ExternalInput")
    bv = nc.dram_tensor("bv", [128, 1024], F32, kind="ExternalInput")
    wp = nc.dram_tensor("wp", [1024, C], F32R, kind="ExternalInput")
    rtab_u = nc.dram_tensor("rtab_u", [128, T], F32, kind="ExternalInput")
    rtab_v = nc.dram_tensor("rtab_v", [128, T], F32, kind="ExternalInput")
    cos4 = nc.dram_tensor("cos4", [T, 256], F32, kind="ExternalInput")
    sin4 = nc.dram_tensor("sin4", [T, 256], F32, kind="ExternalInput")
    masks = nc.dram_tensor("masks", [128, 2048], F32, kind="ExternalInput")
    yT = nc.dram_tensor("yT", [C, T], F32, kind="ExternalOutput")

    qkT = nc.dram_tensor("qkT", [2048, T], F32R)
    vr_d = nc.dram_tensor("vr_d", [T, 1024], F32R)
    oT_d = nc.dram_tensor("oT_d", [1024, T], F32R)

    with tile.TileContext(nc) as tc:
        if True:
            # ---------------- Phase A1: q^T and rope(k)^T ----------------
            # xT resident (128KB/partition); W_qk streamed per column tile.
            with tc.tile_pool(name="xt", bufs=1) as xtp, \
                 tc.tile_pool(name="a1tab", bufs=1) as atabp, \
                 tc.tile_pool(name="wblk", bufs=2) as wbp, \
                 tc.tile_pool(name="qko", bufs=4) as qkop, \
                 tc.tile_pool(name="ktmp", bufs=2) as ktp, \
                 tc.tile_pool(name="psA1", bufs=6, space="PSUM") as psp:
                xt = xtp.tile([128, 16, T], F32R)
                nc.sync.dma_start(xt[:], xT.rearrange("(a p) t -> p a t", p=128))
                ut = atabp.tile([128, T], F32)
                nc.sync.dma_start(ut[:], rtab_u[:, :])
                vt_tab = atabp.tile([128, T], F32)
                nc.sync.dma_start(vt_tab[:], rtab_v[:, :])
                bqk_t = atabp.tile([128, 16], F32)
                nc.sync.dma_start(bqk_t[:], bqk[:, :])

                wqk_r = wqk.rearrange("(a p) j -> p a j", p=128)
                for jt in range(16):
                    wblk = wbp.tile([128, 16, 128], F32R, tag="wblk")
                    nc.sync.dma_start(
                        wblk[:], wqk_r[:, :, jt * 128:(jt + 1) * 128])
                    for tb in range(4):
                        ts = bass.ts(tb, 512)
                        ps = psp.tile([128, 512], F32, tag="ps")
                        for c in range(16):
                            nc.tensor.matmul(
                                ps[:], wblk[:, c], xt[:, c, ts],
                                start=(c == 0), stop=(c == 15))
                        if jt < 8:
                            qo = qkop.tile([128, 512], F32R, tag="qko")
                            nc.vector.tensor_scalar_add(
                                qo[:], ps[:], bqk_t[:, jt:jt + 1])
                            nc.sync.dma_start(
                                qkT[jt * 128:(jt + 1) * 128, ts], qo[:])
                        else:
                            kt = ktp.tile([128, 512], F32, tag="kt")
                            nc.vector.tensor_scalar_add(
                                kt[:], ps[:], bqk_t[:, jt:jt + 1])
                            kts = ktp.tile([128, 512], F32, tag="kts")
                            nc.sync.dma_start(kts[0:64, :], kt[64:128, :])
                            nc.sync.dma_start(kts[64:128, :], kt[0:64, :])
                            m1 = ktp.tile([128, 512], F32, tag="m1")
                            nc.vector.tensor_mul(m1[:], kt[:], ut[:, ts])
                            m2 = ktp.tile([128, 512], F32, tag="m2")
                            nc.vector.tensor_mul(
                                m2[:], kts[:], vt_tab[:, ts])
                            ko = qkop.tile([128, 512], F32R, tag="qko")
                            nc.vector.tensor_add(ko[:], m1[:], m2[:])
                            nc.sync.dma_start(
                                qkT[jt * 128:(jt + 1) * 128, ts], ko[:])

            # ---------------- Phase A2: v natural + rope ----------------
            # Full Wv resident (64KB/partition); xT tiles streamed.
            with tc.tile_pool(name="wvf", bufs=1) as wvp, \
                 tc.tile_pool(name="xa", bufs=3) as xap, \
                 tc.tile_pool(name="vtab", bufs=1) as vtabp, \
                 tc.tile_pool(name="vro", bufs=4) as vrop, \
                 tc.tile_pool(name="vtmp", bufs=2) as vtp, \
                 tc.tile_pool(name="psA2", bufs=6, space="PSUM") as psp:
                wvf = wvp.tile([128, 16, 1024], F32R)
                nc.sync.dma_start(wvf[:], wv.rearrange("(a p) d -> p a d", p=128))
                bv_t = vtabp.tile([128, 1024], F32)
                nc.sync.dma_start(bv_t[:], bv[:, :])
                c4 = vtabp.tile([128, 16, 256], F32)
                nc.sync.dma_start(
                    c4[:], cos4.rearrange("(a p) i -> p a i", p=128))
                s4 = vtabp.tile([128, 16, 256], F32)
                nc.sync.dma_start(
                    s4[:], sin4.rearrange("(a p) i -> p a i", p=128))

                xT_r = xT.rearrange("(a p) t -> p a t", p=128)
                for tt in range(16):
                    xa = xap.tile([128, 16, 128], F32R, tag="xa")
                    nc.sync.dma_start(
                        xa[:], xT_r[:, :, bass.ts(tt, 128)])
                    for db in range(2):
                        ds = bass.ts(db, 512)
                        ps = psp.tile([128, 512], F32, tag="ps")
                        for c in range(16):
                            nc.tensor.matmul(
                                ps[:], xa[:, c],
                                wvf[:, c, ds], start=(c == 0), stop=(c == 15))
                        vtmp = vtp.tile([128, 512], F32, tag="vtmp")
                        nc.vector.tensor_add(vtmp[:], ps[:], bv_t[:, ds])
                        v3 = vtmp[:].rearrange("p (h d) -> p h d", h=4)
                        c43 = c4[:, tt].rearrange("p (h d) -> p h d", h=4)
                        s43 = s4[:, tt].rearrange("p (h d) -> p h d", h=4)
                        me = vtp.tile([128, 4, 64], F32, tag="me")
                        mo = vtp.tile([128, 4, 64], F32, tag="mo")
                        vro = vrop.tile([128, 512], F32R, tag="vro")
                        vr3 = vro[:].rearrange("p (h d) -> p h d", h=4)
                        nc.vector.tensor_mul(
                            me[:], v3[:, :, 0:64], c43[:, :, 0:64])
                        nc.vector.tensor_mul(
                            mo[:], v3[:, :, 64:128], s43[:, :, 0:64])
                        nc.vector.tensor_sub(
                            vr3[:, :, 0:64], me[:], mo[:])
                        nc.vector.tensor_mul(
                            me[:], v3[:, :, 0:64], s43[:, :, 0:64])
                        nc.vector.tensor_mul(
                            mo[:], v3[:, :, 64:128], c43[:, :, 0:64])
                        nc.vector.tensor_add(
                            vr3[:, :, 64:128], me[:], mo[:])
                        nc.sync.dma_start(
                            vr_d[bass.ts(tt, 128), ds], vro[:])

            # ---------------- Phase B: attention per head ----------------
            with tc.tile_pool(name="hk", bufs=2) as hkp, \
                 tc.tile_pool(name="hq", bufs=2) as hqp, \
                 tc.tile_pool(name="hv", bufs=2) as hvp, \
                 tc.tile_pool(name="pt", bufs=4) as ptp, \
                 tc.tile_pool(name="bsc", bufs=2) as bscp, \
                 tc.tile_pool(name="oo", bufs=2) as oop, \
                 tc.tile_pool(name="const", bufs=1) as constp, \
                 tc.tile_pool(name="psB", bufs=5, space="PSUM") as psp, \
                 tc.tile_pool(name="lps", bufs=1, space="PSUM") as lpsp, \
                 tc.tile_pool(name="ops", bufs=2, space="PSUM") as opsp:
                ones_f = constp.tile([128, 1], F32)
                nc.vector.memset(ones_f[:], 1.0)
                ones_t = constp.tile([128, 1], F32R)
                nc.vector.tensor_copy(ones_t[:], ones_f[:])
                masks_t = constp.tile([128, 4, 512], F32)
                nc.sync.dma_start(
                    masks_t[:], masks.rearrange("p (r i) -> p r i", r=4))
                vr_r = vr_d.rearrange("(jt p) d -> p jt d", p=128)
                for h in range(HC):
                    krh = hkp.tile([128, T], F32R, tag="krh")
                    nc.sync.dma_start(
                        krh[:], qkT[1024 + h * 128:1024 + (h + 1) * 128, :])
                    qh = hqp.tile([128, T], F32R, tag="qh")
                    nc.sync.dma_start(qh[:], qkT[h * 128:(h + 1) * 128, :])
                    vh = hvp.tile([128, 16, 128], F32R, tag="vh")
                    nc.sync.dma_start(
                        vh[:], vr_r[:, :, h * 128:(h + 1) * 128])

                    for ib in range(4):
                        isl = bass.ts(ib, 512)
                        nj = 4 * ib + 4
                        l_ps = lpsp.tile([1, 512], F32, tag="l")
                        o_ps = opsp.tile([128, 512], F32, tag="o")
                        pts = [None] * nj

                        def consume(jt):
                            pt = pts[jt]
                            nc.tensor.matmul(
                                l_ps[:], ones_t[:], pt[:],
                                start=(jt == 0), stop=(jt == nj - 1))
                            nc.tensor.matmul(
                                o_ps[:], vh[:, jt], pt[:],
                                start=(jt == 0), stop=(jt == nj - 1))

                        for jt in range(nj):
                            s_ps = psp.tile([128, 512], F32, tag="ps")
                            nc.tensor.matmul(
                                s_ps[:], krh[:, bass.ts(jt, 128)],
                                qh[:, isl], start=True, stop=True)
                            pt = ptp.tile([128, 512], F32R, tag="pt")
                            nc.scalar.activation(
                                pt[:], s_ps[:],
                                mybir.ActivationFunctionType.Exp, scale=SCALE)
                            if jt >= 4 * ib:
                                nc.vector.tensor_mul(
                                    pt[:], pt[:], masks_t[:, jt - 4 * ib])
                            pts[jt] = pt
                            if jt >= 1:
                                consume(jt - 1)
                        consume(nj - 1)

                        r_sb = bscp.tile([1, 512], F32, tag="r")
                        nc.vector.reciprocal(r_sb[:], l_ps[:])
                        rb = bscp.tile([128, 512], F32, tag="rb")
                        nc.gpsimd.partition_broadcast(rb[:], r_sb[:])
                        oo = oop.tile([128, 512], F32R, tag="oo")
                        nc.vector.tensor_mul(oo[:], o_ps[:], rb[:])
                        nc.sync.dma_start(
                            oT_d[h * 128:(h + 1) * 128, isl], oo[:])

            # ---------------- Phase C: out projection ----------------
            with tc.tile_pool(name="wpb", bufs=1) as wpp, \
                 tc.tile_pool(name="otb", bufs=2) as otbp, \
                 tc.tile_pool(name="yo", bufs=4) as yop, \
                 tc.tile_pool(name="psC", bufs=6, space="PSUM") as psp:
                wps = wpp.tile([128, 8, C], F32R)
                nc.sync.dma_start(
                    wps[:], wp.rearrange("(ht p) c -> p ht c", p=128))
                oT_r = oT_d.rearrange("(ht p) t -> p ht t", p=128)
                for tb in range(4):
                    ts = bass.ts(tb, 512)
                    otb = otbp.tile([128, 8, 512], F32R, tag="otb")
                    nc.sync.dma_start(otb[:], oT_r[:, :, ts])
                    for ct in range(16):
                        ps = psp.tile([128, 512], F32, tag="ps")
                        for ht in range(8):
                            nc.tensor.matmul(
                                ps[:], wps[:, ht, bass.ts(ct, 128)],
                                otb[:, ht], start=(ht == 0), stop=(ht == 7))
                        yo = yop.tile([128, 512], F32, tag="yo")
                        nc.vector.tensor_copy(yo[:], ps[:])
                        nc.sync.dma_start(
                            yT[ct * 128:(ct + 1) * 128, ts], yo[:])

    nc.compile()
    return nc


def _prep_inputs(x, freqs_cos, freqs_sin, W_attn, b_attn, W_proj):
    """Host-side sharding / layout prep.  Returns list of 8 in_maps."""
    perm = np.concatenate([np.arange(0, HD, 2), np.arange(1, HD, 2)])

    cosT = np.ascontiguousarray(freqs_cos.T)  # [64, T]
    sinT = np.ascontiguousarray(freqs_sin.T)
    rtab_u = np.concatenate([cosT, cosT], axis=0).astype(np.float32)
    rtab_v = np.concatenate([-sinT, sinT], axis=0).astype(np.float32)
    cos4 = np.tile(freqs_cos, (1, 4)).astype(np.float32)  # [T, 256]
    sin4 = np.tile(freqs_sin, (1, 4)).astype(np.float32)

    jj = np.arange(128)[:, None]
    ii = np.arange(512)[None, :]
    masks = np.concatenate(
        [((r * 128 + jj) <= ii).astype(np.float32) for r in range(4)],
        axis=1)  # [128, 2048]

    in_maps = []
    for core in range(NCORES):
        b = core // 2
        hs = HC * (core % 2)
        cols = np.concatenate(
            [g * HD + perm for g in range(hs, hs + HC)])  # [1024]

        wqk = np.concatenate(
            [W_attn[:, cols], W_attn[:, C + cols]], axis=1)
        bqk_flat = np.concatenate([b_attn[cols], b_attn[C + cols]])
        bqk = np.ascontiguousarray(
            bqk_flat.reshape(16, 128).T)  # [128, 16], bias[jt*128+p]
        wv = W_attn[:, 2 * C + cols]
        bv = np.broadcast_to(b_attn[2 * C + cols], (128, 1024))
        wp = W_proj[cols, :]

        in_maps.append({
            "xT": np.ascontiguousarray(x[b].T).astype(np.float32),
            "wqk": np.ascontiguousarray(wqk).astype(np.float32),
            "bqk": np.ascontiguousarray(bqk).astype(np.float32),
            "wv": np.ascontiguousarray(wv).astype(np.float32),
            "bv": np.ascontiguousarray(bv).astype(np.float32),
            "wp": np.ascontiguousarray(wp).astype(np.float32),
            "rtab_u": rtab_u,
            "rtab_v": rtab_v,
            "cos4": cos4,
            "sin4": sin4,
            "masks": np.ascontiguousarray(masks),
        })
    return in_maps


def kernel(x, freqs_cos, freqs_sin, mask, W_attn, b_attn, W_proj, b_proj,
           _return_results=False):
    x = np.asarray(x, dtype=np.float32)
    freqs_cos = np.asarray(freqs_cos, dtype=np.float32)
    freqs_sin = np.asarray(freqs_sin, dtype=np.float32)
    W_attn = np.asarray(W_attn, dtype=np.float32)
    b_attn = np.asarray(b_attn, dtype=np.float32)
    W_proj = np.asarray(W_proj, dtype=np.float32)
    b_proj = np.asarray(b_proj, dtype=np.float32)

    if "nc" not in _CACHE:
        _CACHE["nc"] = _build_nc()
    nc = _CACHE["nc"]

    in_maps = _prep_inputs(x, freqs_cos, freqs_sin, W_attn, b_attn, W_proj)
    res = run_bass_kernel_spmd(nc, in_maps, core_ids=list(range(NCORES)))

    out = np.empty((B, T, C), dtype=np.float32)
    for b in range(B):
        yt0 = res.results[2 * b]["yT"]
        yt1 = res.results[2 * b + 1]["yT"]
        out[b] = yt0.T + yt1.T + b_proj[None, :]
    if _return_results:
        return out, res
    return out



# revision 3
# speedup vs baseline: 1.1233x; 1.1233x over previous
"""Causal self-attention (RoPE on k/v) TRN2 Bass kernel — bf16 pipeline.

Sharding: core i handles batch b = i//2 and 8 heads hs = 8*(i%2).
Each core computes the qkv projection for its (batch, head-group), RoPE
on k and v, causal attention, and a partial output projection y^T with
its W_proj row-block.  Host sums the two partials per batch, adds
b_proj.

Structure (single TileContext, minimal phase gaps):
  A: qkv projection.  x^T resident in SBUF (bf16, streamed in 4 T-chunks
     so compute starts early).  q^T and rope(k)^T written to a resident
     SBUF tile qkT_sb [128, 16, T] (bf16) — no DRAM round-trip.  rope(v)
     streamed to DRAM (vr, bf16) in natural [T, d] layout.
  B: attention per head.  Scores/PV free dims clipped to the causal
     prefix at 128-key granularity (exact causal work).  Softmax 1/l via
     exp(-ln l) on the Scalar engine (avoids the 3.3us serial DVE
     reciprocal).  Attention out written to resident SBUF o_sb (bf16).
     W_proj prefetched at the start of B.
  C: out projection from o_sb, partial y^T written bf16.

Head-dim permutation (even dims first) turns RoPE's interleaved
even/odd pairs into contiguous 64-row/col halves; W_attn columns and
W_proj rows are permuted correspondingly on host, which leaves the
attention math invariant.

All matmuls in bf16 (1 PE cycle/row at any moving size, enabling the
causal clipping; fp32 PSUM accumulate).
"""
import sys

sys.path.insert(0, "/opt/trn_rl_repo")

import ml_dtypes
import numpy as np

import concourse.bass as bass  # noqa: F401
import concourse.mybir as mybir
import concourse.tile as tile
from concourse import bacc
from concourse.bass_utils import run_bass_kernel_spmd

B, T, C, H = 4, 2048, 2048, 16
HD = 128
HC = 8  # heads per core
NCORES = 8
F32 = mybir.dt.float32
BF16 = mybir.dt.bfloat16
BF16_NP = ml_dtypes.bfloat16
SCALE = float(1.0 / np.sqrt(HD))

_CACHE = {}


def _build_nc():
    nc = bacc.Bacc(num_devices=NCORES)

    xT = nc.dram_tensor("xT", [C, T], BF16, kind="ExternalInput")
    wqk = nc.dram_tensor("wqk", [C, 2048], BF16, kind="ExternalInput")
    bqk = nc.dram_tensor("bqk", [128, 16], F32, kind="ExternalInput")
    wv = nc.dram_tensor("wv", [C, 1024], BF16, kind="ExternalInput")
    bv = nc.dram_tensor("bv", [128, 1024], F32, kind="ExternalInput")
    wp = nc.dram_tensor("wp", [1024, C], BF16, kind="ExternalInput")
    rtab_u = nc.dram_tensor("rtab_u", [128, T], BF16, kind="ExternalInput")
    rtab_v = nc.dram_tensor("rtab_v", [128, T], BF16, kind="ExternalInput")
    cos4 = nc.dram_tensor("cos4", [T, 256], BF16, kind="ExternalInput")
    sin4 = nc.dram_tensor("sin4", [T, 256], BF16, kind="ExternalInput")
    mask1 = nc.dram_tensor("mask1", [128, 128], BF16, kind="ExternalInput")
    yT = nc.dram_tensor("yT", [C, T], BF16, kind="ExternalOutput")

    vr_d = nc.dram_tensor("vr_d", [T, 1024], BF16)

    with tile.TileContext(nc) as tc:
        with tc.tile_pool(name="tabs", bufs=1) as tabp, \
             tc.tile_pool(name="qksb", bufs=1) as qkp:
            # ---- persistent: resident qk + small tables ----
            qkT_sb = qkp.tile([128, 16, T], BF16)
            bqk_t = tabp.tile([128, 16], F32)
            m1_t = tabp.tile([128, 128], BF16)
            ones_f = tabp.tile([128, 1], F32)
            ones_b = tabp.tile([128, 1], BF16)

            # ------------- Phase A: qkv projection + rope -------------
            with tc.tile_pool(name="atab", bufs=1) as atabp, \
                 tc.tile_pool(name="xt", bufs=1) as xtp, \
                 tc.tile_pool(name="wvf", bufs=1) as wvp, \
                 tc.tile_pool(name="wblk", bufs=2) as wbp, \
                 tc.tile_pool(name="cs", bufs=2) as csp, \
                 tc.tile_pool(name="ktmp", bufs=2) as ktp, \
                 tc.tile_pool(name="vtmp", bufs=2) as vtp, \
                 tc.tile_pool(name="vro", bufs=3) as vrop, \
                 tc.tile_pool(name="psA", bufs=5, space="PSUM") as psp:
                xt = xtp.tile([128, 16, T], BF16)
                xT_r = xT.rearrange("(a p) t -> p a t", p=128)
                # stream x in 4 T-chunks so the first matmul starts early
                for tc4 in range(4):
                    nc.sync.dma_start(
                        xt[:, :, bass.ts(tc4, 512)],
                        xT_r[:, :, bass.ts(tc4, 512)])
                wqk_r = wqk.rearrange("(a p) j -> p a j", p=128)
                wblk0 = wbp.tile([128, 16, 128], BF16, tag="wblk")
                nc.sync.dma_start(wblk0[:], wqk_r[:, :, 0:128])
                nc.sync.dma_start(bqk_t[:], bqk[:, :])
                ut = atabp.tile([128, T], BF16)
                nc.sync.dma_start(ut[:], rtab_u[:, :])
                vt_tab = atabp.tile([128, T], BF16)
                nc.sync.dma_start(vt_tab[:], rtab_v[:, :])
                wvf = wvp.tile([128, 16, 1024], BF16)
                nc.sync.dma_start(wvf[:], wv.rearrange("(a p) d -> p a d", p=128))
                bv_t = atabp.tile([128, 1024], F32)
                nc.sync.dma_start(bv_t[:], bv[:, :])
                nc.sync.dma_start(m1_t[:], mask1[:, :])
                nc.vector.memset(ones_f[:], 1.0)
                nc.vector.tensor_copy(ones_b[:], ones_f[:])
                cos4_r = cos4.rearrange("(a p) i -> p a i", p=128)
                sin4_r = sin4.rearrange("(a p) i -> p a i", p=128)

                # ---- A-qk: q^T and rope(k)^T into qkT_sb ----
                wblk = wblk0
                for jt in range(16):
                    if jt + 1 < 16:
                        wblk_n = wbp.tile([128, 16, 128], BF16, tag="wblk")
                        nc.sync.dma_start(
                            wblk_n[:],
                            wqk_r[:, :, (jt + 1) * 128:(jt + 2) * 128])
                    for tb in range(4):
                        ts = bass.ts(tb, 512)
                        ps = psp.tile([128, 512], F32, tag="ps")
                        for c in range(16):
                            nc.tensor.matmul(
                                ps[:], wblk[:, c], xt[:, c, ts],
                                start=(c == 0), stop=(c == 15))
                        if jt < 8:
                            # q: bias add on the Scalar engine, bf16 out
                            nc.scalar.activation(
                                qkT_sb[:, jt, ts], ps[:],
                                mybir.ActivationFunctionType.Identity,
                                bias=bqk_t[:, jt:jt + 1])
                        else:
                            # k: bias + rope on DVE (swap halves via DMA)
                            kt = ktp.tile([128, 512], BF16, tag="kt")
                            nc.vector.tensor_scalar_add(
                                kt[:], ps[:], bqk_t[:, jt:jt + 1])
                            kts = ktp.tile([128, 512], BF16, tag="kts")
                            nc.sync.dma_start(kts[0:64, :], kt[64:128, :])
                            nc.sync.dma_start(kts[64:128, :], kt[0:64, :])
                            m1 = ktp.tile([128, 512], BF16, tag="m1")
                            nc.vector.tensor_mul(m1[:], kt[:], ut[:, ts])
                            m2 = ktp.tile([128, 512], BF16, tag="m2")
                            nc.vector.tensor_mul(
                                m2[:], kts[:], vt_tab[:, ts])
                            nc.vector.tensor_add(
                                qkT_sb[:, jt, ts], m1[:], m2[:])
                    wblk = wblk_n if jt + 1 < 16 else None

                # ---- A-v: rope(v) natural layout -> DRAM ----
                for tt in range(16):
                    c4t = csp.tile([128, 256], BF16, tag="c4")
                    nc.sync.dma_start(c4t[:], cos4_r[:, tt, :])
                    s4t = csp.tile([128, 256], BF16, tag="s4")
                    nc.sync.dma_start(s4t[:], sin4_r[:, tt, :])
                    c43 = c4t[:].rearrange("p (h d) -> p h d", h=4)
                    s43 = s4t[:].rearrange("p (h d) -> p h d", h=4)
                    for db in range(2):
                        ds = bass.ts(db, 512)
                        ps = psp.tile([128, 512], F32, tag="ps")
                        for c in range(16):
                            nc.tensor.matmul(
                                ps[:], xt[:, c, bass.ts(tt, 128)],
                                wvf[:, c, ds], start=(c == 0), stop=(c == 15))
                        vb = vtp.tile([128, 512], BF16, tag="vb")
                        nc.vector.tensor_add(vb[:], ps[:], bv_t[:, ds])
                        v3 = vb[:].rearrange("p (h d) -> p h d", h=4)
                        me = vtp.tile([128, 4, 64], BF16, tag="me")
                        mo = vtp.tile([128, 4, 64], BF16, tag="mo")
                        vro = vrop.tile([128, 512], BF16, tag="vro")
                        vr3 = vro[:].rearrange("p (h d) -> p h d", h=4)
                        nc.vector.tensor_mul(
                            me[:], v3[:, :, 0:64], c43[:, :, 0:64])
                        nc.vector.tensor_mul(
                            mo[:], v3[:, :, 64:128], s43[:, :, 0:64])
                        nc.vector.tensor_sub(
                            vr3[:, :, 0:64], me[:], mo[:])
                        nc.vector.tensor_mul(
                            me[:], v3[:, :, 0:64], s43[:, :, 0:64])
                        nc.vector.tensor_mul(
                            mo[:], v3[:, :, 64:128], c43[:, :, 0:64])
                        nc.vector.tensor_add(
                            vr3[:, :, 64:128], me[:], mo[:])
                        nc.sync.dma_start(
                            vr_d[bass.ts(tt, 128), ds], vro[:])

            # ------------- Phases B + C -------------
            with tc.tile_pool(name="osb", bufs=1) as osbp, \
                 tc.tile_pool(name="wpb", bufs=1) as wpp:
                o_sb = osbp.tile([128, HC, T], BF16)
                wps = wpp.tile([128, 8, C], BF16)
                # prefetch W_proj under phase-B compute
                nc.sync.dma_start(
                    wps[:], wp.rearrange("(ht p) c -> p ht c", p=128))

                with tc.tile_pool(name="hv", bufs=2) as hvp, \
                     tc.tile_pool(name="pt", bufs=6) as ptp, \
                     tc.tile_pool(name="bsc", bufs=3) as bscp, \
                     tc.tile_pool(name="psS", bufs=3, space="PSUM") as psp, \
                     tc.tile_pool(name="lps", bufs=2, space="PSUM") as lpsp, \
                     tc.tile_pool(name="ops", bufs=2, space="PSUM") as opsp:
                    vr_r = vr_d.rearrange("(jt p) d -> p jt d", p=128)
                    vh = hvp.tile([128, 16, 128], BF16, tag="vh")
                    nc.sync.dma_start(vh[:], vr_r[:, :, 0:128])
                    for h in range(HC):
                        if h + 1 < HC:
                            vh_n = hvp.tile([128, 16, 128], BF16, tag="vh")
                            nc.sync.dma_start(
                                vh_n[:],
                                vr_r[:, :, (h + 1) * 128:(h + 2) * 128])

                        for ib in (3, 2, 1, 0):
                            isl = bass.ts(ib, 512)
                            nj = 4 * ib + 4
                            l_ps = lpsp.tile([1, 512], F32, tag="l")
                            o_ps = opsp.tile([128, 512], F32, tag="o")
                            pts = [None] * nj
                            qss = [max(0, 128 * jt - 512 * ib)
                                   for jt in range(nj)]

                            def consume(jt, vh=vh, l_ps=l_ps, o_ps=o_ps,
                                        pts=pts, qss=qss, nj=nj):
                                pt = pts[jt]
                                qs = qss[jt]
                                nc.tensor.matmul(
                                    l_ps[:, qs:], ones_b[:], pt[:, qs:],
                                    start=(jt == 0), stop=(jt == nj - 1))
                                nc.tensor.matmul(
                                    o_ps[:, qs:], vh[:, jt], pt[:, qs:],
                                    start=(jt == 0), stop=(jt == nj - 1))

                            for jt in range(nj):
                                qs = qss[jt]
                                s_ps = psp.tile([128, 512], F32, tag="ps")
                                nc.tensor.matmul(
                                    s_ps[:, qs:],
                                    qkT_sb[:, 8 + h, bass.ts(jt, 128)],
                                    qkT_sb[:, h, 512 * ib + qs:512 * (ib + 1)],
                                    start=True, stop=True)
                                pt = ptp.tile([128, 512], BF16, tag="pt")
                                nc.scalar.activation(
                                    pt[:, qs:], s_ps[:, qs:],
                                    mybir.ActivationFunctionType.Exp,
                                    scale=SCALE)
                                if jt >= 4 * ib:
                                    nc.vector.tensor_mul(
                                        pt[:, qs:qs + 128],
                                        pt[:, qs:qs + 128], m1_t[:])
                                pts[jt] = pt
                                if jt >= 1:
                                    consume(jt - 1)
                            consume(nj - 1)

                            # 1/l = exp(-ln l) on the Scalar engine
                            lnl = bscp.tile([1, 512], F32, tag="lnl")
                            nc.scalar.activation(
                                lnl[:], l_ps[:],
                                mybir.ActivationFunctionType.Ln)
                            r_sb = bscp.tile([1, 512], F32, tag="r")
                            nc.scalar.activation(
                                r_sb[:], lnl[:],
                                mybir.ActivationFunctionType.Exp, scale=-1.0)
                            rb = bscp.tile([128, 512], F32, tag="rb")
                            nc.gpsimd.partition_broadcast(rb[:], r_sb[:])
                            nc.vector.tensor_mul(
                                o_sb[:, h, isl], o_ps[:], rb[:])
                        vh = vh_n if h + 1 < HC else None

                # ------------- Phase C: out projection -------------
                with tc.tile_pool(name="yo", bufs=4) as yop, \
                     tc.tile_pool(name="psC", bufs=4, space="PSUM") as pscp:
                    for tb in range(4):
                        ts = bass.ts(tb, 512)
                        for ct in range(16):
                            ps = pscp.tile([128, 512], F32, tag="ps")
                            for ht in range(8):
                                nc.tensor.matmul(
                                    ps[:], wps[:, ht, bass.ts(ct, 128)],
                                    o_sb[:, ht, ts],
                                    start=(ht == 0), stop=(ht == 7))
                            yo = yop.tile([128, 512], BF16, tag="yo")
                            nc.scalar.activation(
                                yo[:], ps[:],
                                mybir.ActivationFunctionType.Copy)
                            nc.sync.dma_start(
                                yT[ct * 128:(ct + 1) * 128, ts], yo[:])

    nc.compile()
    return nc


def _prep_inputs(x, freqs_cos, freqs_sin, W_attn, b_attn, W_proj):
    """Host-side sharding / layout prep.  Returns list of 8 in_maps."""
    perm = np.concatenate([np.arange(0, HD, 2), np.arange(1, HD, 2)])

    cosT = np.ascontiguousarray(freqs_cos.T)  # [64, T]
    sinT = np.ascontiguousarray(freqs_sin.T)
    rtab_u = np.concatenate([cosT, cosT], axis=0).astype(BF16_NP)
    rtab_v = np.concatenate([-sinT, sinT], axis=0).astype(BF16_NP)
    cos4 = np.tile(freqs_cos, (1, 4)).astype(BF16_NP)  # [T, 256]
    sin4 = np.tile(freqs_sin, (1, 4)).astype(BF16_NP)

    jj = np.arange(128)[:, None]
    ii = np.arange(128)[None, :]
    mask1 = (jj <= ii).astype(BF16_NP)  # [128, 128]

    in_maps = []
    for core in range(NCORES):
        b = core // 2
        hs = HC * (core % 2)
        cols = np.concatenate(
            [g * HD + perm for g in range(hs, hs + HC)])  # [1024]

        wqk = np.concatenate(
            [W_attn[:, cols], W_attn[:, C + cols]], axis=1)
        bqk_flat = np.concatenate([b_attn[cols], b_attn[C + cols]])
        bqk = np.ascontiguousarray(
            bqk_flat.reshape(16, 128).T)  # [128, 16], bias[jt*128+p]
        wv = W_attn[:, 2 * C + cols]
        bv = np.broadcast_to(b_attn[2 * C + cols], (128, 1024))
        wp = W_proj[cols, :]

        in_maps.append({
            "xT": np.ascontiguousarray(x[b].T).astype(BF16_NP),
            "wqk": np.ascontiguousarray(wqk).astype(BF16_NP),
            "bqk": np.ascontiguousarray(bqk).astype(np.float32),
            "wv": np.ascontiguousarray(wv).astype(BF16_NP),
            "bv": np.ascontiguousarray(bv).astype(np.float32),
            "wp": np.ascontiguousarray(wp).astype(BF16_NP),
            "rtab_u": rtab_u,
            "rtab_v": rtab_v,
            "cos4": cos4,
            "sin4": sin4,
            "mask1": np.ascontiguousarray(mask1),
        })
    return in_maps


def kernel(x, freqs_cos, freqs_sin, mask, W_attn, b_attn, W_proj, b_proj,
           _return_results=False):
    x = np.asarray(x, dtype=np.float32)
    freqs_cos = np.asarray(freqs_cos, dtype=np.float32)
    freqs_sin = np.asarray(freqs_sin, dtype=np.float32)
    W_attn = np.asarray(W_attn, dtype=np.float32)
    b_attn = np.asarray(b_attn, dtype=np.float32)
    W_proj = np.asarray(W_proj, dtype=np.float32)
    b_proj = np.asarray(b_proj, dtype=np.float32)

    if "nc" not in _CACHE:
        _CACHE["nc"] = _build_nc()
    nc = _CACHE["nc"]

    in_maps = _prep_inputs(x, freqs_cos, freqs_sin, W_attn, b_attn, W_proj)
    res = run_bass_kernel_spmd(nc, in_maps, core_ids=list(range(NCORES)))

    out = np.empty((B, T, C), dtype=np.float32)
    for b in range(B):
        yt0 = res.results[2 * b]["yT"].astype(np.float32)
        yt1 = res.results[2 * b + 1]["yT"].astype(np.float32)
        out[b] = yt0.T + yt1.T + b_proj[None, :]
    if _return_results:
        return out, res
    return out


# revision 8
# speedup vs baseline: 1.2091x; 1.0764x over previous
"""Causal self-attention (RoPE on k/v) TRN2 Bass kernel — bf16 pipeline.

Sharding: core i handles batch b = i//2 and 8 heads hs = 8*(i%2).
Each core computes the qkv projection for its (batch, head-group), RoPE
on k and v, causal attention, and a partial output projection y^T with
its W_proj row-block.  Host sums the two partials per batch, adds
b_proj.

Structure (single TileContext, minimal phase gaps):
  A: qkv projection.  x^T resident in SBUF (bf16, streamed in 4 T-chunks
     so compute starts early).  q^T and rope(k)^T written to a resident
     SBUF tile qkT_sb [128, 16, T] (bf16) — no DRAM round-trip.  rope(v)
     streamed to DRAM (vr, bf16) in natural [T, d] layout.
  B: attention per head.  Scores/PV free dims clipped to the causal
     prefix at 128-key granularity (exact causal work).  Softmax 1/l via
     exp(-ln l) on the Scalar engine (avoids the 3.3us serial DVE
     reciprocal).  Attention out written to resident SBUF o_sb (bf16).
     W_proj prefetched at the start of B.
  C: out projection from o_sb, partial y^T written bf16.

Head-dim permutation (even dims first) turns RoPE's interleaved
even/odd pairs into contiguous 64-row/col halves; W_attn columns and
W_proj rows are permuted correspondingly on host, which leaves the
attention math invariant.

All matmuls in bf16 (1 PE cycle/row at any moving size, enabling the
causal clipping; fp32 PSUM accumulate).
"""
import sys

sys.path.insert(0, "/opt/trn_rl_repo")

import ml_dtypes
import numpy as np

import concourse.bass as bass  # noqa: F401
import concourse.mybir as mybir
import concourse.tile as tile
from concourse import bacc
from concourse.bass_utils import run_bass_kernel_spmd

B, T, C, H = 4, 2048, 2048, 16
HD = 128
HC = 8  # heads per core
NCORES = 8
F32 = mybir.dt.float32
BF16 = mybir.dt.bfloat16
BF16_NP = ml_dtypes.bfloat16
SCALE = float(1.0 / np.sqrt(HD))

_CACHE = {}


def _build_nc():
    nc = bacc.Bacc(num_devices=NCORES)

    xT = nc.dram_tensor("xT", [C, T], BF16, kind="ExternalInput")
    wqk = nc.dram_tensor("wqk", [C, 2048], BF16, kind="ExternalInput")
    bqk = nc.dram_tensor("bqk", [128, 16], F32, kind="ExternalInput")
    wv = nc.dram_tensor("wv", [C, 1024], BF16, kind="ExternalInput")
    bv = nc.dram_tensor("bv", [128, 1024], F32, kind="ExternalInput")
    wp = nc.dram_tensor("wp", [1024, C], BF16, kind="ExternalInput")
    rtab_u = nc.dram_tensor("rtab_u", [128, T], BF16, kind="ExternalInput")
    rtab_v = nc.dram_tensor("rtab_v", [128, T], BF16, kind="ExternalInput")
    cos4 = nc.dram_tensor("cos4", [T, 256], BF16, kind="ExternalInput")
    sin4 = nc.dram_tensor("sin4", [T, 256], BF16, kind="ExternalInput")
    mask1 = nc.dram_tensor("mask1", [128, 128], BF16, kind="ExternalInput")
    yT = nc.dram_tensor("yT", [C, T], BF16, kind="ExternalOutput")

    vr_d = nc.dram_tensor("vr_d", [T, 1024], BF16)

    with tile.TileContext(nc) as tc:
        with tc.tile_pool(name="tabs", bufs=1) as tabp, \
             tc.tile_pool(name="qksb", bufs=1) as qkp:
            # ---- persistent: resident qk + small tables ----
            qkT_sb = qkp.tile([128, 16, T], BF16)
            bqk_t = tabp.tile([128, 16], F32)
            m1_t = tabp.tile([128, 128], BF16)
            ones_f = tabp.tile([128, 1], F32)
            ones_b = tabp.tile([128, 1], BF16)

            # ------------- Phase A: qkv projection + rope -------------
            with tc.tile_pool(name="atab", bufs=1) as atabp, \
                 tc.tile_pool(name="xt", bufs=1) as xtp, \
                 tc.tile_pool(name="wvf", bufs=1) as wvp, \
                 tc.tile_pool(name="wblk", bufs=2) as wbp, \
                 tc.tile_pool(name="cs", bufs=2) as csp, \
                 tc.tile_pool(name="ktmp", bufs=3) as ktp, \
                 tc.tile_pool(name="vtmp", bufs=3) as vtp, \
                 tc.tile_pool(name="vro", bufs=3) as vrop, \
                 tc.tile_pool(name="psA", bufs=5, space="PSUM") as psp:
                xt = xtp.tile([128, 16, T], BF16)
                xT_r = xT.rearrange("(a p) t -> p a t", p=128)
                # first T-chunk of x and the first weight block lead the
                # DMA issue order so the first matmul chain starts early
                nc.sync.dma_start(
                    xt[:, :, 0:512], xT_r[:, :, 0:512])
                wqk_r = wqk.rearrange("(a p) j -> p a j", p=128)
                wblk0 = wbp.tile([128, 16, 128], BF16, tag="wblk")
                nc.sync.dma_start(wblk0[:], wqk_r[:, :, 0:128])
                nc.sync.dma_start(bqk_t[:], bqk[:, :])
                for tc4 in range(1, 4):
                    nc.sync.dma_start(
                        xt[:, :, bass.ts(tc4, 512)],
                        xT_r[:, :, bass.ts(tc4, 512)])
                ut = atabp.tile([128, T], BF16)
                nc.sync.dma_start(ut[:], rtab_u[:, :])
                vt_tab = atabp.tile([128, T], BF16)
                nc.sync.dma_start(vt_tab[:], rtab_v[:, :])
                wvf = wvp.tile([128, 16, 1024], BF16)
                wv_r = wv.rearrange("(a p) d -> p a d", p=128)
                for wc in range(4):
                    nc.sync.dma_start(
                        wvf[:, bass.ts(wc, 4), :], wv_r[:, bass.ts(wc, 4), :])
                bv_t = atabp.tile([128, 1024], F32)
                nc.sync.dma_start(bv_t[:], bv[:, :])
                nc.sync.dma_start(m1_t[:], mask1[:, :])
                nc.vector.memset(ones_f[:], 1.0)
                nc.vector.tensor_copy(ones_b[:], ones_f[:])
                cos4_r = cos4.rearrange("(a p) i -> p a i", p=128)
                sin4_r = sin4.rearrange("(a p) i -> p a i", p=128)

                # ---- A-qk: q^T and rope(k)^T into qkT_sb ----
                wblk = wblk0
                for jt in range(16):
                    if jt + 1 < 16:
                        wblk_n = wbp.tile([128, 16, 128], BF16, tag="wblk")
                        nc.sync.dma_start(
                            wblk_n[:],
                            wqk_r[:, :, (jt + 1) * 128:(jt + 2) * 128])
                    for tb in range(4):
                        ts = bass.ts(tb, 512)
                        ps = psp.tile([128, 512], F32, tag="ps")
                        for c in range(16):
                            nc.tensor.matmul(
                                ps[:], wblk[:, c], xt[:, c, ts],
                                start=(c == 0), stop=(c == 15))
                        if jt < 8:
                            # q: bias add on the Scalar engine, bf16 out
                            nc.scalar.activation(
                                qkT_sb[:, jt, ts], ps[:],
                                mybir.ActivationFunctionType.Identity,
                                bias=bqk_t[:, jt:jt + 1])
                        else:
                            # k: bias + rope on DVE (swap halves via DMA)
                            kt = ktp.tile([128, 512], BF16, tag="kt")
                            nc.vector.tensor_scalar_add(
                                kt[:], ps[:], bqk_t[:, jt:jt + 1])
                            kts = ktp.tile([128, 512], BF16, tag="kts")
                            nc.sync.dma_start(kts[0:64, :], kt[64:128, :])
                            nc.sync.dma_start(kts[64:128, :], kt[0:64, :])
                            m1 = ktp.tile([128, 512], BF16, tag="m1")
                            nc.vector.tensor_mul(m1[:], kt[:], ut[:, ts])
                            m2 = ktp.tile([128, 512], BF16, tag="m2")
                            nc.vector.tensor_mul(
                                m2[:], kts[:], vt_tab[:, ts])
                            nc.vector.tensor_add(
                                qkT_sb[:, jt, ts], m1[:], m2[:])
                    wblk = wblk_n if jt + 1 < 16 else None

                # ---- A-v: rope(v) natural layout -> DRAM ----
                for tt in range(16):
                    c4t = csp.tile([128, 256], BF16, tag="c4")
                    nc.sync.dma_start(c4t[:], cos4_r[:, tt, :])
                    s4t = csp.tile([128, 256], BF16, tag="s4")
                    nc.sync.dma_start(s4t[:], sin4_r[:, tt, :])
                    c43 = c4t[:].rearrange("p (h d) -> p h d", h=4)
                    s43 = s4t[:].rearrange("p (h d) -> p h d", h=4)
                    for db in range(2):
                        ds = bass.ts(db, 512)
                        ps = psp.tile([128, 512], F32, tag="ps")
                        for c in range(16):
                            nc.tensor.matmul(
                                ps[:], xt[:, c, bass.ts(tt, 128)],
                                wvf[:, c, ds], start=(c == 0), stop=(c == 15))
                        vb = vtp.tile([128, 512], BF16, tag="vb")
                        nc.vector.tensor_add(vb[:], ps[:], bv_t[:, ds])
                        v3 = vb[:].rearrange("p (h d) -> p h d", h=4)
                        me = vtp.tile([128, 4, 64], BF16, tag="me")
                        mo = vtp.tile([128, 4, 64], BF16, tag="mo")
                        vro = vrop.tile([128, 512], BF16, tag="vro")
                        vr3 = vro[:].rearrange("p (h d) -> p h d", h=4)
                        nc.vector.tensor_mul(
                            me[:], v3[:, :, 0:64], c43[:, :, 0:64])
                        nc.vector.tensor_mul(
                            mo[:], v3[:, :, 64:128], s43[:, :, 0:64])
                        nc.vector.tensor_sub(
                            vr3[:, :, 0:64], me[:], mo[:])
                        nc.vector.tensor_mul(
                            me[:], v3[:, :, 0:64], s43[:, :, 0:64])
                        nc.vector.tensor_mul(
                            mo[:], v3[:, :, 64:128], c43[:, :, 0:64])
                        nc.vector.tensor_add(
                            vr3[:, :, 64:128], me[:], mo[:])
                        nc.sync.dma_start(
                            vr_d[bass.ts(tt, 128), ds], vro[:])

            # ------------- Phases B + C -------------
            with tc.tile_pool(name="osb", bufs=1) as osbp, \
                 tc.tile_pool(name="wpb", bufs=1) as wpp:
                o_sb = osbp.tile([128, HC, T], BF16)
                wps = wpp.tile([128, 8, C], BF16)
                # prefetch W_proj under phase-B compute (split for parallelism)
                wp_r = wp.rearrange("(ht p) c -> p ht c", p=128)
                for ht in range(8):
                    nc.sync.dma_start(wps[:, ht, :], wp_r[:, ht, :])

                with tc.tile_pool(name="hv", bufs=2) as hvp, \
                     tc.tile_pool(name="pt", bufs=6) as ptp, \
                     tc.tile_pool(name="bsc", bufs=3) as bscp, \
                     tc.tile_pool(name="psS", bufs=3, space="PSUM") as psp, \
                     tc.tile_pool(name="lps", bufs=2, space="PSUM") as lpsp, \
                     tc.tile_pool(name="ops", bufs=2, space="PSUM") as opsp:
                    vr_r = vr_d.rearrange("(jt p) d -> p jt d", p=128)

                    def load_vh(hh):
                        # split by key-chunk so each piece only depends on
                        # the matching vr store (overlaps phase-A tail)
                        t_ = hvp.tile([128, 16, 128], BF16, tag="vh")
                        for kc in range(4):
                            nc.sync.dma_start(
                                t_[:, bass.ts(kc, 4), :],
                                vr_r[:, bass.ts(kc, 4),
                                     hh * 128:(hh + 1) * 128])
                        return t_

                    vh = load_vh(0)
                    for h in range(HC):
                        if h + 1 < HC:
                            vh_n = load_vh(h + 1)

                        for ib in (3, 2, 1, 0):
                            isl = bass.ts(ib, 512)
                            nj = 4 * ib + 4
                            l_ps = lpsp.tile([1, 512], F32, tag="l")
                            o_ps = opsp.tile([128, 512], F32, tag="o")
                            pts = [None] * nj
                            qss = [max(0, 128 * jt - 512 * ib)
                                   for jt in range(nj)]

                            def consume(jt, vh=vh, l_ps=l_ps, o_ps=o_ps,
                                        pts=pts, qss=qss, nj=nj):
                                pt = pts[jt]
                                qs = qss[jt]
                                nc.tensor.matmul(
                                    l_ps[:, qs:], ones_b[:], pt[:, qs:],
                                    start=(jt == 0), stop=(jt == nj - 1))
                                nc.tensor.matmul(
                                    o_ps[:, qs:], vh[:, jt], pt[:, qs:],
                                    start=(jt == 0), stop=(jt == nj - 1))

                            for jt in range(nj):
                                qs = qss[jt]
                                s_ps = psp.tile([128, 512], F32, tag="ps")
                                nc.tensor.matmul(
                                    s_ps[:, qs:],
                                    qkT_sb[:, 8 + h, bass.ts(jt, 128)],
                                    qkT_sb[:, h, 512 * ib + qs:512 * (ib + 1)],
                                    start=True, stop=True)
                                pt = ptp.tile([128, 512], BF16, tag="pt")
                                nc.scalar.activation(
                                    pt[:, qs:], s_ps[:, qs:],
                                    mybir.ActivationFunctionType.Exp,
                                    scale=SCALE)
                                if jt >= 4 * ib:
                                    nc.vector.tensor_mul(
                                        pt[:, qs:qs + 128],
                                        pt[:, qs:qs + 128], m1_t[:])
                                pts[jt] = pt
                                if jt >= 1:
                                    consume(jt - 1)
                            consume(nj - 1)

                            # 1/l: bf16 reciprocal on DVE (2x path), then
                            # broadcast + multiply
                            l_sb = bscp.tile([1, 512], BF16, tag="lsb")
                            nc.vector.tensor_copy(l_sb[:], l_ps[:])
                            r_sb = bscp.tile([1, 512], BF16, tag="r")
                            with nc.allow_low_precision(
                                    "softmax denom recip in bf16"):
                                nc.vector.reciprocal(r_sb[:], l_sb[:])
                            rb = bscp.tile([128, 512], BF16, tag="rb")
                            nc.gpsimd.partition_broadcast(rb[:], r_sb[:])
                            nc.vector.tensor_mul(
                                o_sb[:, h, isl], o_ps[:], rb[:])
                        vh = vh_n if h + 1 < HC else None

                # ------------- Phase C: out projection -------------
                with tc.tile_pool(name="yo", bufs=4) as yop, \
                     tc.tile_pool(name="psC", bufs=4, space="PSUM") as pscp:
                    for tb in range(4):
                        ts = bass.ts(tb, 512)
                        for ct in range(16):
                            ps = pscp.tile([128, 512], F32, tag="ps")
                            for ht in range(8):
                                nc.tensor.matmul(
                                    ps[:], wps[:, ht, bass.ts(ct, 128)],
                                    o_sb[:, ht, ts],
                                    start=(ht == 0), stop=(ht == 7))
                            yo = yop.tile([128, 512], BF16, tag="yo")
                            nc.scalar.activation(
                                yo[:], ps[:],
                                mybir.ActivationFunctionType.Copy)
                            nc.sync.dma_start(
                                yT[ct * 128:(ct + 1) * 128, ts], yo[:])

    nc.compile()
    return nc


def _prep_inputs(x, freqs_cos, freqs_sin, W_attn, b_attn, W_proj):
    """Host-side sharding / layout prep.  Returns list of 8 in_maps."""
    perm = np.concatenate([np.arange(0, HD, 2), np.arange(1, HD, 2)])

    cosT = np.ascontiguousarray(freqs_cos.T)  # [64, T]
    sinT = np.ascontiguousarray(freqs_sin.T)
    rtab_u = np.concatenate([cosT, cosT], axis=0).astype(BF16_NP)
    rtab_v = np.concatenate([-sinT, sinT], axis=0).astype(BF16_NP)
    cos4 = np.tile(freqs_cos, (1, 4)).astype(BF16_NP)  # [T, 256]
    sin4 = np.tile(freqs_sin, (1, 4)).astype(BF16_NP)

    jj = np.arange(128)[:, None]
    ii = np.arange(128)[None, :]
    mask1 = (jj <= ii).astype(BF16_NP)  # [128, 128]

    in_maps = []
    for core in range(NCORES):
        b = core // 2
        hs = HC * (core % 2)
        cols = np.concatenate(
            [g * HD + perm for g in range(hs, hs + HC)])  # [1024]

        wqk = np.concatenate(
            [W_attn[:, cols], W_attn[:, C + cols]], axis=1)
        bqk_flat = np.concatenate([b_attn[cols], b_attn[C + cols]])
        bqk = np.ascontiguousarray(
            bqk_flat.reshape(16, 128).T)  # [128, 16], bias[jt*128+p]
        wv = W_attn[:, 2 * C + cols]
        bv = np.broadcast_to(b_attn[2 * C + cols], (128, 1024))
        wp = W_proj[cols, :]

        in_maps.append({
            "xT": np.ascontiguousarray(x[b].T).astype(BF16_NP),
            "wqk": np.ascontiguousarray(wqk).astype(BF16_NP),
            "bqk": np.ascontiguousarray(bqk).astype(np.float32),
            "wv": np.ascontiguousarray(wv).astype(BF16_NP),
            "bv": np.ascontiguousarray(bv).astype(np.float32),
            "wp": np.ascontiguousarray(wp).astype(BF16_NP),
            "rtab_u": rtab_u,
            "rtab_v": rtab_v,
            "cos4": cos4,
            "sin4": sin4,
            "mask1": np.ascontiguousarray(mask1),
        })
    return in_maps


def kernel(x, freqs_cos, freqs_sin, mask, W_attn, b_attn, W_proj, b_proj,
           _return_results=False):
    x = np.asarray(x, dtype=np.float32)
    freqs_cos = np.asarray(freqs_cos, dtype=np.float32)
    freqs_sin = np.asarray(freqs_sin, dtype=np.float32)
    W_attn = np.asarray(W_attn, dtype=np.float32)
    b_attn = np.asarray(b_attn, dtype=np.float32)
    W_proj = np.asarray(W_proj, dtype=np.float32)
    b_proj = np.asarray(b_proj, dtype=np.float32)

    if "nc" not in _CACHE:
        _CACHE["nc"] = _build_nc()
    nc = _CACHE["nc"]

    in_maps = _prep_inputs(x, freqs_cos, freqs_sin, W_attn, b_attn, W_proj)
    res = run_bass_kernel_spmd(nc, in_maps, core_ids=list(range(NCORES)))

    out = np.empty((B, T, C), dtype=np.float32)
    for b in range(B):
        yt0 = res.results[2 * b]["yT"].astype(np.float32)
        yt1 = res.results[2 * b + 1]["yT"].astype(np.float32)
        out[b] = yt0.T + yt1.T + b_proj[None, :]
    if _return_results:
        return out, res
    return out


# revision 19
# speedup vs baseline: 1.2286x; 1.0161x over previous
"""Causal self-attention (RoPE on k/v) TRN2 Bass kernel — bf16 pipeline.

Sharding: core i handles batch b = i//2 and 8 heads hs = 8*(i%2).
Each core computes the qkv projection for its (batch, head-group), RoPE
on k and v, causal attention, and a partial output projection y^T with
its W_proj row-block.  Host sums the two partials per batch, adds
b_proj.

Structure (single TileContext, minimal phase gaps):
  A: qkv projection.  x^T resident in SBUF (bf16, streamed in 4 T-chunks
     so compute starts early).  q^T and rope(k)^T written to a resident
     SBUF tile qkT_sb [128, 16, T] (bf16) — no DRAM round-trip.  rope(v)
     streamed to DRAM (vr, bf16) in natural [T, d] layout.
  B: attention per head.  Scores/PV free dims clipped to the causal
     prefix at 128-key granularity (exact causal work).  Softmax 1/l via
     exp(-ln l) on the Scalar engine (avoids the 3.3us serial DVE
     reciprocal).  Attention out written to resident SBUF o_sb (bf16).
     W_proj prefetched at the start of B.
  C: out projection from o_sb, partial y^T written bf16.

Head-dim permutation (even dims first) turns RoPE's interleaved
even/odd pairs into contiguous 64-row/col halves; W_attn columns and
W_proj rows are permuted correspondingly on host, which leaves the
attention math invariant.

All matmuls in bf16 (1 PE cycle/row at any moving size, enabling the
causal clipping; fp32 PSUM accumulate).
"""
import sys

sys.path.insert(0, "/opt/trn_rl_repo")

import ml_dtypes
import numpy as np

import concourse.bass as bass  # noqa: F401
import concourse.mybir as mybir
import concourse.tile as tile
from concourse import bacc
from concourse.bass_utils import run_bass_kernel_spmd

B, T, C, H = 4, 2048, 2048, 16
HD = 128
HC = 8  # heads per core
NCORES = 8
F32 = mybir.dt.float32
BF16 = mybir.dt.bfloat16
BF16_NP = ml_dtypes.bfloat16
SCALE = float(1.0 / np.sqrt(HD))

_CACHE = {}


def _build_nc():
    nc = bacc.Bacc(num_devices=NCORES)

    xT = nc.dram_tensor("xT", [C, T], BF16, kind="ExternalInput")
    wqk = nc.dram_tensor("wqk", [C, 2048], BF16, kind="ExternalInput")
    bqk = nc.dram_tensor("bqk", [128, 16], F32, kind="ExternalInput")
    wv = nc.dram_tensor("wv", [C, 1024], BF16, kind="ExternalInput")
    bv = nc.dram_tensor("bv", [128, 1024], F32, kind="ExternalInput")
    wp = nc.dram_tensor("wp", [1024, C], BF16, kind="ExternalInput")
    rtab_u = nc.dram_tensor("rtab_u", [128, T], BF16, kind="ExternalInput")
    rtab_v = nc.dram_tensor("rtab_v", [128, T], BF16, kind="ExternalInput")
    cos4 = nc.dram_tensor("cos4", [T, 256], BF16, kind="ExternalInput")
    sin4 = nc.dram_tensor("sin4", [T, 256], BF16, kind="ExternalInput")
    mask1 = nc.dram_tensor("mask1", [128, 128], BF16, kind="ExternalInput")
    yT = nc.dram_tensor("yT", [C, T], BF16, kind="ExternalOutput")

    vr_d = nc.dram_tensor("vr_d", [T, 1024], BF16)

    with tile.TileContext(nc) as tc:
        with tc.tile_pool(name="tabs", bufs=1) as tabp, \
             tc.tile_pool(name="qksb", bufs=1) as qkp, \
             tc.tile_pool(name="hv", bufs=2) as hvp:
            # ---- persistent: resident qk + small tables ----
            qkT_sb = qkp.tile([128, 16, T], BF16)
            bqk_t = tabp.tile([128, 16], F32)
            m1_t = tabp.tile([128, 128], BF16)
            ones_f = tabp.tile([128, 128], F32)
            ones_b = tabp.tile([128, 128], BF16)
            vr_r = vr_d.rearrange("(jt p) d -> p jt d", p=128)

            def load_vh_kc(t_, hh, kc):
                nc.sync.dma_start(
                    t_[:, bass.ts(kc, 4), :],
                    vr_r[:, bass.ts(kc, 4), hh * 128:(hh + 1) * 128])

            def load_vh(hh):
                t_ = hvp.tile([128, 16, 128], BF16, tag="vh")
                for kc in range(4):
                    load_vh_kc(t_, hh, kc)
                return t_

            # ------------- Phase A: qkv projection + rope -------------
            with tc.tile_pool(name="atab", bufs=1) as atabp, \
                 tc.tile_pool(name="xt", bufs=1) as xtp, \
                 tc.tile_pool(name="wvf", bufs=1) as wvp, \
                 tc.tile_pool(name="wblk", bufs=2) as wbp, \
                 tc.tile_pool(name="cs", bufs=2) as csp, \
                 tc.tile_pool(name="ktmp", bufs=2) as ktp, \
                 tc.tile_pool(name="vtmp", bufs=2) as vtp, \
                 tc.tile_pool(name="vro", bufs=2) as vrop, \
                 tc.tile_pool(name="psA", bufs=5, space="PSUM") as psp:
                xt = xtp.tile([128, 16, T], BF16)
                xT_r = xT.rearrange("(a p) t -> p a t", p=128)
                # first T-chunks of x and the first weight block lead the
                # DMA issue order so the first matmul chains start early
                for sub in range(2):
                    nc.sync.dma_start(
                        xt[:, bass.ts(sub, 8), 0:512],
                        xT_r[:, bass.ts(sub, 8), 0:512])
                wqk_r = wqk.rearrange("(a p) j -> p a j", p=128)
                wblk0 = wbp.tile([128, 16, 128], BF16, tag="wblk")
                nc.sync.dma_start(wblk0[:], wqk_r[:, :, 0:128])
                for sub in range(2):
                    nc.sync.dma_start(
                        xt[:, bass.ts(sub, 8), 512:1024],
                        xT_r[:, bass.ts(sub, 8), 512:1024])
                nc.sync.dma_start(bqk_t[:], bqk[:, :])
                for tc4 in range(2, 4):
                    nc.sync.dma_start(
                        xt[:, :, bass.ts(tc4, 512)],
                        xT_r[:, :, bass.ts(tc4, 512)])
                ut = atabp.tile([128, T], BF16)
                nc.sync.dma_start(ut[:], rtab_u[:, :])
                vt_tab = atabp.tile([128, T], BF16)
                nc.sync.dma_start(vt_tab[:], rtab_v[:, :])
                wvf = wvp.tile([128, 16, 1024], BF16)
                wv_r = wv.rearrange("(a p) d -> p a d", p=128)
                for wc in range(4):
                    nc.sync.dma_start(
                        wvf[:, bass.ts(wc, 4), :], wv_r[:, bass.ts(wc, 4), :])
                bv_t = atabp.tile([128, 1024], F32)
                nc.sync.dma_start(bv_t[:], bv[:, :])
                nc.sync.dma_start(m1_t[:], mask1[:, :])
                nc.vector.memset(ones_f[:], 1.0)
                nc.vector.tensor_copy(ones_b[:], ones_f[:])
                cos4_r = cos4.rearrange("(a p) i -> p a i", p=128)
                sin4_r = sin4.rearrange("(a p) i -> p a i", p=128)

                # ---- A-qk: q^T and rope(k)^T into qkT_sb ----
                wblk = wblk0
                for jt in range(16):
                    if jt + 1 < 16:
                        wblk_n = wbp.tile([128, 16, 128], BF16, tag="wblk")
                        nc.sync.dma_start(
                            wblk_n[:],
                            wqk_r[:, :, (jt + 1) * 128:(jt + 2) * 128])
                    for tb in range(4):
                        ts = bass.ts(tb, 512)
                        ps = psp.tile([128, 512], F32, tag="ps")
                        for c in range(16):
                            nc.tensor.matmul(
                                ps[:], wblk[:, c], xt[:, c, ts],
                                start=(c == 0), stop=(c == 15))
                        if jt < 8:
                            # q: bias add on the Scalar engine, bf16 out
                            nc.scalar.activation(
                                qkT_sb[:, jt, ts], ps[:],
                                mybir.ActivationFunctionType.Identity,
                                bias=bqk_t[:, jt:jt + 1])
                        else:
                            # k: bias + rope on DVE (swap halves via DMA)
                            kt = ktp.tile([128, 512], BF16, tag="kt")
                            nc.vector.tensor_scalar_add(
                                kt[:], ps[:], bqk_t[:, jt:jt + 1])
                            kts = ktp.tile([128, 512], BF16, tag="kts")
                            nc.sync.dma_start(kts[0:64, :], kt[64:128, :])
                            nc.sync.dma_start(kts[64:128, :], kt[0:64, :])
                            m1 = ktp.tile([128, 512], BF16, tag="m1")
                            nc.vector.tensor_mul(m1[:], kt[:], ut[:, ts])
                            m2 = ktp.tile([128, 512], BF16, tag="m2")
                            nc.vector.tensor_mul(
                                m2[:], kts[:], vt_tab[:, ts])
                            nc.vector.tensor_add(
                                qkT_sb[:, jt, ts], m1[:], m2[:])
                    wblk = wblk_n if jt + 1 < 16 else None

                # ---- A-v: rope(v) natural layout -> DRAM ----
                vh0 = hvp.tile([128, 16, 128], BF16, tag="vh")
                vh1 = hvp.tile([128, 16, 128], BF16, tag="vh")
                for tt in range(16):
                    c4t = csp.tile([128, 256], BF16, tag="c4")
                    nc.sync.dma_start(c4t[:], cos4_r[:, tt, :])
                    s4t = csp.tile([128, 256], BF16, tag="s4")
                    nc.sync.dma_start(s4t[:], sin4_r[:, tt, :])
                    c43 = c4t[:].rearrange("p (h d) -> p h d", h=4)
                    s43 = s4t[:].rearrange("p (h d) -> p h d", h=4)
                    for db in range(2):
                        ds = bass.ts(db, 512)
                        ps = psp.tile([128, 512], F32, tag="ps")
                        for c in range(16):
                            nc.tensor.matmul(
                                ps[:], xt[:, c, bass.ts(tt, 128)],
                                wvf[:, c, ds], start=(c == 0), stop=(c == 15))
                        vb = vtp.tile([128, 512], BF16, tag="vb")
                        nc.vector.tensor_add(vb[:], ps[:], bv_t[:, ds])
                        v3 = vb[:].rearrange("p (h d) -> p h d", h=4)
                        me = vtp.tile([128, 4, 64], BF16, tag="me")
                        mo = vtp.tile([128, 4, 64], BF16, tag="mo")
                        vro = vrop.tile([128, 512], BF16, tag="vro")
                        vr3 = vro[:].rearrange("p (h d) -> p h d", h=4)
                        nc.vector.tensor_mul(
                            me[:], v3[:, :, 0:64], c43[:, :, 0:64])
                        nc.vector.tensor_mul(
                            mo[:], v3[:, :, 64:128], s43[:, :, 0:64])
                        nc.vector.tensor_sub(
                            vr3[:, :, 0:64], me[:], mo[:])
                        nc.vector.tensor_mul(
                            me[:], v3[:, :, 0:64], s43[:, :, 0:64])
                        nc.vector.tensor_mul(
                            mo[:], v3[:, :, 64:128], c43[:, :, 0:64])
                        nc.vector.tensor_add(
                            vr3[:, :, 64:128], me[:], mo[:])
                        nc.sync.dma_start(
                            vr_d[bass.ts(tt, 128), ds], vro[:])
                    if tt % 4 == 3:
                        # prefetch first two heads' v as stores complete
                        load_vh_kc(vh0, 0, tt // 4)
                        load_vh_kc(vh1, 1, tt // 4)

            # ------------- Phases B + C -------------
            with tc.tile_pool(name="osb", bufs=1) as osbp, \
                 tc.tile_pool(name="wpb", bufs=1) as wpp:
                o_sb = osbp.tile([128, HC, T], BF16)
                wps = wpp.tile([128, 8, C], BF16)
                # prefetch W_proj under phase-B compute (split for parallelism)
                wp_r = wp.rearrange("(ht p) c -> p ht c", p=128)
                for ht in range(8):
                    nc.sync.dma_start(wps[:, ht, :], wp_r[:, ht, :])

                with tc.tile_pool(name="hv", bufs=2) as hvp, \
                     tc.tile_pool(name="pt", bufs=6) as ptp, \
                     tc.tile_pool(name="bsc", bufs=3) as bscp, \
                     tc.tile_pool(name="psS", bufs=3, space="PSUM") as psp, \
                     tc.tile_pool(name="lps", bufs=2, space="PSUM") as lpsp, \
                     tc.tile_pool(name="ops", bufs=2, space="PSUM") as opsp:
                    vhs = [vh0, vh1]
                    for h in range(HC):
                        vh = vhs[h]
                        for ib in (3, 2, 1, 0):
                            isl = bass.ts(ib, 512)
                            nj = 4 * ib + 4
                            l_ps = lpsp.tile([1, 512], F32, tag="l")
                            o_ps = opsp.tile([128, 512], F32, tag="o")
                            pts = [None] * nj
                            qss = [max(0, 128 * jt - 512 * ib)
                                   for jt in range(nj)]

                            def consume(jt, vh=vh, l_ps=l_ps, o_ps=o_ps,
                                        pts=pts, qss=qss, nj=nj):
                                pt = pts[jt]
                                qs = qss[jt]
                                nc.tensor.matmul(
                                    l_ps[:, qs:], ones_b[:], pt[:, qs:],
                                    start=(jt == 0), stop=(jt == nj - 1))
                                nc.tensor.matmul(
                                    o_ps[:, qs:], vh[:, jt], pt[:, qs:],
                                    start=(jt == 0), stop=(jt == nj - 1))

                            for jt in range(nj):
                                qs = qss[jt]
                                s_ps = psp.tile([128, 512], F32, tag="ps")
                                nc.tensor.matmul(
                                    s_ps[:, qs:],
                                    qkT_sb[:, 8 + h, bass.ts(jt, 128)],
                                    qkT_sb[:, h, 512 * ib + qs:512 * (ib + 1)],
                                    start=True, stop=True)
                                pt = ptp.tile([128, 512], BF16, tag="pt")
                                nc.scalar.activation(
                                    pt[:, qs:], s_ps[:, qs:],
                                    mybir.ActivationFunctionType.Exp,
                                    scale=SCALE)
                                if jt >= 4 * ib:
                                    nc.vector.tensor_mul(
                                        pt[:, qs:qs + 128],
                                        pt[:, qs:qs + 128], m1_t[:])
                                pts[jt] = pt
                                # 2-tile lookahead: pt[jt-2] has had time to
                                # flow through exp (Scalar) + mask (DVE)
                                if jt >= 2:
                                    consume(jt - 2)
                            if nj >= 2:
                                consume(nj - 2)
                            consume(nj - 1)

                            # 1/l: bf16 reciprocal on DVE (2x path), then
                            # broadcast + multiply
                            l_sb = bscp.tile([1, 512], BF16, tag="lsb")
                            nc.vector.tensor_copy(l_sb[:], l_ps[:])
                            r_sb = bscp.tile([1, 512], BF16, tag="r")
                            with nc.allow_low_precision(
                                    "softmax denom recip in bf16"):
                                nc.vector.reciprocal(r_sb[:], l_sb[:])
                            rb = bscp.tile([128, 512], BF16, tag="rb")
                            nc.gpsimd.partition_broadcast(rb[:], r_sb[:])
                            nc.vector.tensor_mul(
                                o_sb[:, h, isl], o_ps[:], rb[:])
                        if h + 2 < HC:
                            vhs.append(load_vh(h + 2))
                        else:
                            vhs.append(None)

                # ------------- Phase C: out projection -------------
                with tc.tile_pool(name="yo", bufs=4) as yop, \
                     tc.tile_pool(name="psC", bufs=4, space="PSUM") as pscp:
                    for tb in range(4):
                        ts = bass.ts(tb, 512)
                        for ct in range(16):
                            ps = pscp.tile([128, 512], F32, tag="ps")
                            for ht in range(8):
                                nc.tensor.matmul(
                                    ps[:], wps[:, ht, bass.ts(ct, 128)],
                                    o_sb[:, ht, ts],
                                    start=(ht == 0), stop=(ht == 7))
                            yo = yop.tile([128, 512], BF16, tag="yo")
                            nc.scalar.activation(
                                yo[:], ps[:],
                                mybir.ActivationFunctionType.Copy)
                            nc.sync.dma_start(
                                yT[ct * 128:(ct + 1) * 128, ts], yo[:])

    nc.compile()
    return nc


def _prep_inputs(x, freqs_cos, freqs_sin, W_attn, b_attn, W_proj):
    """Host-side sharding / layout prep.  Returns list of 8 in_maps."""
    perm = np.concatenate([np.arange(0, HD, 2), np.arange(1, HD, 2)])

    cosT = np.ascontiguousarray(freqs_cos.T)  # [64, T]
    sinT = np.ascontiguousarray(freqs_sin.T)
    rtab_u = np.concatenate([cosT, cosT], axis=0).astype(BF16_NP)
    rtab_v = np.concatenate([-sinT, sinT], axis=0).astype(BF16_NP)
    cos4 = np.tile(freqs_cos, (1, 4)).astype(BF16_NP)  # [T, 256]
    sin4 = np.tile(freqs_sin, (1, 4)).astype(BF16_NP)

    jj = np.arange(128)[:, None]
    ii = np.arange(128)[None, :]
    mask1 = (jj <= ii).astype(BF16_NP)  # [128, 128]

    in_maps = []
    for core in range(NCORES):
        b = core // 2
        hs = HC * (core % 2)
        cols = np.concatenate(
            [g * HD + perm for g in range(hs, hs + HC)])  # [1024]

        wqk = np.concatenate(
            [W_attn[:, cols], W_attn[:, C + cols]], axis=1)
        bqk_flat = np.concatenate([b_attn[cols], b_attn[C + cols]])
        bqk = np.ascontiguousarray(
            bqk_flat.reshape(16, 128).T)  # [128, 16], bias[jt*128+p]
        wv = W_attn[:, 2 * C + cols]
        bv = np.broadcast_to(b_attn[2 * C + cols], (128, 1024))
        wp = W_proj[cols, :]

        in_maps.append({
            "xT": np.ascontiguousarray(x[b].T).astype(BF16_NP),
            "wqk": np.ascontiguousarray(wqk).astype(BF16_NP),
            "bqk": np.ascontiguousarray(bqk).astype(np.float32),
            "wv": np.ascontiguousarray(wv).astype(BF16_NP),
            "bv": np.ascontiguousarray(bv).astype(np.float32),
            "wp": np.ascontiguousarray(wp).astype(BF16_NP),
            "rtab_u": rtab_u,
            "rtab_v": rtab_v,
            "cos4": cos4,
            "sin4": sin4,
            "mask1": np.ascontiguousarray(mask1),
        })
    return in_maps


def kernel(x, freqs_cos, freqs_sin, mask, W_attn, b_attn, W_proj, b_proj,
           _return_results=False):
    x = np.asarray(x, dtype=np.float32)
    freqs_cos = np.asarray(freqs_cos, dtype=np.float32)
    freqs_sin = np.asarray(freqs_sin, dtype=np.float32)
    W_attn = np.asarray(W_attn, dtype=np.float32)
    b_attn = np.asarray(b_attn, dtype=np.float32)
    W_proj = np.asarray(W_proj, dtype=np.float32)
    b_proj = np.asarray(b_proj, dtype=np.float32)

    if "nc" not in _CACHE:
        _CACHE["nc"] = _build_nc()
    nc = _CACHE["nc"]

    in_maps = _prep_inputs(x, freqs_cos, freqs_sin, W_attn, b_attn, W_proj)
    res = run_bass_kernel_spmd(nc, in_maps, core_ids=list(range(NCORES)))

    out = np.empty((B, T, C), dtype=np.float32)
    for b in range(B):
        yt0 = res.results[2 * b]["yT"].astype(np.float32)
        yt1 = res.results[2 * b + 1]["yT"].astype(np.float32)
        out[b] = yt0.T + yt1.T + b_proj[None, :]
    if _return_results:
        return out, res
    return out
